# revision 1
# baseline (speedup 1.0000x reference)
"""BalancedTopkMLP Trainium2 kernel: token-parallel across 8 NeuronCores.

reference:
  pred = sigmoid((x @ w_pred1.T) @ w_pred2.T)          [N, I]
  mask = per-bank (128ch) top-16 of |pred|+bias, binary  (bias == 0 here)
  out  = (mask*pred * silu(x@w_gate.T) * (x@w_up.T)) @ w_down.T

Sharding: tokens (B*S = 8192) split 8 ways; each core runs the full MLP on
its 1024 tokens with full weights (no collectives). Host transposes/pre-tiles
weights and splits activations/predictor weights into bf16 hi/lo pairs.

Numerics: gate/up/down in bf16 (fp32 PSUM accumulate). Predictor matmuls use
a 3-term bf16 split (x_h*w_h + x_h*w_l + x_l*w_h, ~4e-6 rel err) so the
per-bank top-16 selection on z matches the fp32 reference's ordering except
for genuinely near-tied scores. Selection runs on pre-sigmoid z (monotone).
"""
import sys
import os
import numpy as np
import ml_dtypes

for _p in ("/opt/trn_rl_repo", os.path.expanduser("~/.axon_site/_ro/trn_rl_repo")):
    if os.path.isdir(_p) and _p not in sys.path:
        sys.path.insert(0, _p)

import concourse.bass as bass  # noqa: E402
import concourse.mybir as mybir  # noqa: E402
from concourse import bacc  # noqa: E402
from concourse.bass_utils import run_bass_kernel_spmd  # noqa: E402
from concourse.tile import TileContext  # noqa: E402
from concourse.masks import make_identity  # noqa: E402

BF16 = mybir.dt.bfloat16
F32R = mybir.dt.float32r
FP32 = mybir.dt.float32
AF = mybir.ActivationFunctionType
OP = mybir.AluOpType

H = 4096
I = 11008
PD = 1024
BANK = 128
TOPK = 16
NB = I // BANK          # 86
NCORES = 8
NTOK_TOT = 8192
NTOK = NTOK_TOT // NCORES   # 1024 per core
BLK = 512                   # tokens per block
NBLK = NTOK // BLK          # 2
CB = 4                      # banks per chunk
NCHUNK = (NB + CB - 1) // CB  # 22 (21x4 + 1x2)
KT_H = H // 128             # 32
KT_P = PD // 128            # 8
KQ = 2                      # phase-1 k-tiles per streamed quarter
NQ = KT_H // KQ             # phase-1 quarters
NEG = -1.0e30

_CACHE = {}


def _chunk_banks(ci):
    b0 = ci * CB
    return b0, min(CB, NB - b0)


def _build():
    nc = bacc.Bacc("TRN2", target_bir_lowering=False, debug=False,
                   num_devices=NCORES)

    def din(name, shape, dt):
        return nc.declare_dram_parameter(name, list(shape), dt, isOutput=False)

    xTh_d = din("xTh", [128, KT_H, NTOK], BF16)
    xr_d = din("xr", [128, KT_H, 2, NTOK], F32R)      # f32r hi/lo pieces
    w1_d = din("w1", [128, KT_H, 2, PD], F32R)
    w2_d = din("w2", [128, KT_P, 2, I], F32R)
    wgu_d = din("wgu", [NB, 128, KT_H, 2, BANK], BF16)  # gate|up strips
    wd_d = din("wd", [H // 512, NB, 128, 512], BF16)  # [hc, k, 128, 512]
    out_d = nc.declare_dram_parameter("out", [NTOK, H], FP32, isOutput=True)

    from contextlib import ExitStack
    with TileContext(nc) as tc, ExitStack() as es:
        ep = es.enter_context
        constp = ep(tc.tile_pool(name="const", bufs=1))
        dramp = ep(tc.tile_pool(name="dram", bufs=NBLK, space="DRAM"))
        xap = ep(tc.tile_pool(name="xa", bufs=1))
        xlp = ep(tc.tile_pool(name="xl", bufs=2))
        xpp = ep(tc.tile_pool(name="xp", bufs=1))
        w1p = ep(tc.tile_pool(name="w1", bufs=3))
        w2p = ep(tc.tile_pool(name="w2", bufs=2))
        wgup = ep(tc.tile_pool(name="wgu", bufs=3))
        zcp = ep(tc.tile_pool(name="zc", bufs=2))
        selp = ep(tc.tile_pool(name="sel", bufs=1))
        m8p = ep(tc.tile_pool(name="m8", bufs=8))
        prp = ep(tc.tile_pool(name="pr", bufs=2))
        mtp = ep(tc.tile_pool(name="mt", bufs=2))
        gup = ep(tc.tile_pool(name="gu", bufs=1))
        htcp = ep(tc.tile_pool(name="htc", bufs=1))
        dnp = ep(tc.tile_pool(name="dn", bufs=2))
        osp = ep(tc.tile_pool(name="os", bufs=2))
        mmps = ep(tc.tile_pool(name="mmps", bufs=3, space="PSUM"))
        trps = ep(tc.tile_pool(name="trps", bufs=1, space="PSUM"))
        dnps = ep(tc.tile_pool(name="dnps", bufs=4, space="PSUM"))

        ident = constp.tile([128, 128], BF16)
        make_identity(nc, ident)

        for blk in range(NBLK):
            t0 = blk * BLK
            # ---- stage x hi for this block (resident through phase 2) ----
            xh = xap.tile([128, KT_H, BLK], BF16, tag="xh")
            nc.sync.dma_start(xh[:], xTh_d[:, :, t0:t0 + BLK])

            # ---- phase 1: xpT = w_pred1 @ x.T  (3-term f32r split, fp32 acc)
            # mi in groups of 4 so each streamed x quarter feeds 4 psum banks
            xph = xpp.tile([128, KT_P, BLK], F32R, tag="xph")
            xpl = xpp.tile([128, KT_P, BLK], F32R, tag="xpl")
            for grp in range(KT_P // 2):
                pts1 = [mmps.tile([128, BLK], FP32, tag="mm", name=f"p1_{j}")
                        for j in range(2)]
                for q in range(NQ):
                    k0 = q * KQ
                    xrt = xlp.tile([128, KQ, 2, BLK], F32R, tag="xr")
                    nc.gpsimd.dma_start(xrt[:], xr_d[:, k0:k0 + KQ, :, t0:t0 + BLK])
                    for j in range(2):
                        mi = grp * 2 + j
                        w1t = w1p.tile([128, KQ, 2, 128], F32R, tag="w1")
                        nc.sync.dma_start(
                            w1t[:], w1_d[:, k0:k0 + KQ, :, mi * 128:(mi + 1) * 128])
                        for k in range(KQ):
                            for i, (l, r) in enumerate((
                                    (w1t[:, k, 0, :], xrt[:, k, 0, :]),
                                    (w1t[:, k, 0, :], xrt[:, k, 1, :]),
                                    (w1t[:, k, 1, :], xrt[:, k, 0, :]))):
                                nc.tensor.matmul(
                                    pts1[j][:], l, r,
                                    start=(q == 0 and k == 0 and i == 0),
                                    stop=(q == NQ - 1 and k == KQ - 1 and i == 2))
                for j in range(2):
                    mi = grp * 2 + j
                    # hi piece (f32r rne-12 round on write) and exact residual
                    nc.scalar.activation(xph[:, mi, :], pts1[j][:], AF.Copy)
                    nc.vector.tensor_tensor(xpl[:, mi, :], pts1[j][:],
                                            xph[:, mi, :].bitcast(FP32),
                                            OP.subtract)

            # DRAM stash for hT of this block
            hts = dramp.tile([128, NB, BLK], BF16, tag="hts")

            # ---- phase 2: chunk loop over I ----
            for ci in range(NCHUNK):
                b0, nb = _chunk_banks(ci)
                c0, cw = b0 * BANK, nb * BANK

                # pred2 -> z chunk [128tok, tt, cw] fp32; w2 streamed in
                # 256-ch halves (double-buffered) so matmuls start early
                z = zcp.tile([128, BLK // 128, CB * BANK], FP32, tag="z")
                nhalf = (cw + 255) // 256
                w2ts = []
                for hf in range(nhalf):
                    hw = min(256, cw - hf * 256)
                    w2t = w2p.tile([128, KT_P, 2, 256], F32R, tag="w2",
                                   name=f"w2_{hf}")
                    nc.sync.dma_start(
                        w2t[:, :, :, :hw],
                        w2_d[:, :, :, c0 + hf * 256:c0 + hf * 256 + hw])
                    w2ts.append((w2t, hw))
                for tt in range(BLK // 128):
                    tsl = slice(tt * 128, (tt + 1) * 128)
                    pt = mmps.tile([128, CB * BANK], FP32, tag="mm")
                    for hf, (w2t, hw) in enumerate(w2ts):
                        osl = slice(hf * 256, hf * 256 + hw)
                        terms = []
                        for mi in range(KT_P):
                            terms.append((xph[:, mi, tsl], w2t[:, mi, 0, :hw]))
                        for mi in range(KT_P):
                            terms.append((xpl[:, mi, tsl], w2t[:, mi, 0, :hw]))
                        for mi in range(KT_P):
                            terms.append((xph[:, mi, tsl], w2t[:, mi, 1, :hw]))
                        for i, (l, r) in enumerate(terms):
                            nc.tensor.matmul(pt[:, osl], l, r,
                                             start=(i == 0),
                                             stop=(i == len(terms) - 1))
                    nc.scalar.activation(z[:, tt, :cw], pt[:, :cw], AF.Copy)

                # selection: top-16 per bank -> zap
                zap = selp.tile([128, BLK // 128, CB * BANK], FP32, tag="zap")
                for tt in range(BLK // 128):
                    for b in range(nb):
                        zin = z[:, tt, b * BANK:(b + 1) * BANK]
                        zzap = zap[:, tt, b * BANK:(b + 1) * BANK]
                        m8 = m8p.tile([128, 8], FP32, tag="m8")
                        nc.vector.max(m8[:], zin)
                        nc.vector.match_replace(zzap, in_to_replace=m8[:],
                                                in_values=zin, imm_value=NEG)
                        m8b = m8p.tile([128, 8], FP32, tag="m8")
                        nc.vector.max(m8b[:], zzap)
                        nc.vector.match_replace(zzap, in_to_replace=m8b[:],
                                                in_values=zzap, imm_value=NEG)
                # mask01 (into zap) = (z != zap); pred = sigmoid(z);
                # masked pred (into pred) = mask01 * pred
                pred = prp.tile([128, BLK // 128, CB * BANK], BF16, tag="pred")
                nc.scalar.activation(pred[:, :, :cw], z[:, :, :cw], AF.Sigmoid)
                nc.vector.tensor_tensor(zap[:, :, :cw], z[:, :, :cw],
                                        zap[:, :, :cw], OP.not_equal)
                nc.vector.tensor_tensor(pred[:, :, :cw], zap[:, :, :cw],
                                        pred[:, :, :cw], OP.mult)

                # transpose masked pred tiles -> mpT [128ch, b, tok]
                mpT = mtp.tile([128, CB, BLK], BF16, tag="mpT")
                for tt in range(BLK // 128):
                    for b in range(nb):
                        tp = trps.tile([128, 128], BF16, tag="tr")
                        nc.tensor.transpose(
                            tp[:], pred[:, tt, b * BANK:(b + 1) * BANK], ident[:])
                        nc.scalar.activation(
                            mpT[:, b, tt * 128:(tt + 1) * 128], tp[:], AF.Copy)

                # gate / up for this chunk's I-tiles
                sg = gup.tile([128, CB, BLK], BF16, tag="sg")
                uu = gup.tile([128, CB, BLK], BF16, tag="uu")
                KHH = KT_H // 2
                for b in range(nb):
                    it = b0 + b
                    wg0 = wgup.tile([128, KHH, 2, BANK], BF16, tag="wgu")
                    nc.gpsimd.dma_start(wg0[:], wgu_d[it, :, :KHH])
                    wg1 = wgup.tile([128, KHH, 2, BANK], BF16, tag="wgu")
                    nc.gpsimd.dma_start(wg1[:], wgu_d[it, :, KHH:])
                    pt = mmps.tile([128, BLK], FP32, tag="mm")
                    for k in range(KT_H):
                        wt = wg0 if k < KHH else wg1
                        nc.tensor.matmul(pt[:], wt[:, k % KHH, 0, :], xh[:, k, :],
                                         start=(k == 0), stop=(k == KT_H - 1))
                    nc.scalar.activation(sg[:, b, :], pt[:], AF.Silu)
                    pt2 = mmps.tile([128, BLK], FP32, tag="mm")
                    for k in range(KT_H):
                        wt = wg0 if k < KHH else wg1
                        nc.tensor.matmul(pt2[:], wt[:, k % KHH, 1, :], xh[:, k, :],
                                         start=(k == 0), stop=(k == KT_H - 1))
                    nc.scalar.activation(uu[:, b, :], pt2[:], AF.Copy)

                # hT chunk = mpT * silu(gate) * up  -> DRAM stash
                ht = htcp.tile([128, CB, BLK], BF16, tag="ht")
                nc.vector.tensor_tensor(ht[:, :nb, :], mpT[:, :nb, :],
                                        sg[:, :nb, :], OP.mult)
                nc.vector.tensor_tensor(ht[:, :nb, :], ht[:, :nb, :],
                                        uu[:, :nb, :], OP.mult)
                nc.sync.dma_start(hts[:, b0:b0 + nb, :], ht[:, :nb, :])

            # ---- phase 3: out = hT.T @ w_downT ----
            for hc in range(H // 512):
                pts = [dnps.tile([128, 512], FP32, tag="dn", name=f"dn_{tt}")
                       for tt in range(BLK // 128)]
                for k2 in range(NB // 2):
                    wdt = dnp.tile([128, 2, 512], BF16, tag="wd")
                    nc.sync.dma_start(
                        wdt[:], wd_d[hc, 2 * k2:2 * k2 + 2].rearrange(
                            "k p n -> p k n"))
                    htt = dnp.tile([128, 2, BLK], BF16, tag="htt")
                    nc.gpsimd.dma_start(htt[:], hts[:, 2 * k2:2 * k2 + 2, :])
                    for kk in range(2):
                        k = 2 * k2 + kk
                        for tt in range(BLK // 128):
                            nc.tensor.matmul(pts[tt][:],
                                             htt[:, kk, tt * 128:(tt + 1) * 128],
                                             wdt[:, kk, :], start=(k == 0),
                                             stop=(k == NB - 1))
                for tt in range(BLK // 128):
                    ot = osp.tile([128, 512], FP32, tag="os")
                    nc.scalar.activation(ot[:], pts[tt][:], AF.Copy)
                    nc.sync.dma_start(
                        out_d[t0 + tt * 128:t0 + (tt + 1) * 128,
                              hc * 512:(hc + 1) * 512], ot[:])

    nc.compile()
    return nc


def _rne12(a):
    """float32r rounding: round-to-nearest-even keeping 11 explicit mantissa
    bits (drops 12 low bits), as measured on TRN2 via identity matmul."""
    v = np.ascontiguousarray(a, np.float32).view(np.uint32)
    add = np.uint32((1 << 11) - 1)
    lsb = (v >> np.uint32(12)) & np.uint32(1)
    return ((v + add + lsb) & np.uint32(0xFFFFF000)).view(np.float32)


def _split_r(a):
    h = _rne12(a)
    return h, (a - h)  # residual is f32r-exact (<= 12 significant bits)


def _prep_inputs(x, w_pred1, w_pred2, w_gate, w_up, w_down):
    bf = ml_dtypes.bfloat16

    def split(a):
        h = a.astype(bf)
        l = (a - h.astype(np.float32)).astype(bf)
        return h, l

    def tile_kxn(a, kt):  # [K, N] -> [128, kt, N]
        K, N = a.shape
        return np.ascontiguousarray(
            a.reshape(kt, 128, N).transpose(1, 0, 2))

    w1h, w1l = _split_r(w_pred1.T.copy())       # [H, PD] f32r pieces
    w2h, w2l = _split_r(w_pred2.T.copy())       # [PD, I]

    def hl(a, b, kt):  # [K,N]x2 -> [128, kt, 2, N]
        K, N = a.shape
        s = np.stack([a.reshape(kt, 128, N), b.reshape(kt, 128, N)], axis=2)
        return np.ascontiguousarray(s.transpose(1, 0, 2, 3))

    shared = {
        "w1": hl(w1h, w1l, KT_H),
        "w2": hl(w2h, w2l, KT_P),
        # wgu: [NB, 128p(H), KT_H, 2, BANK]
        "wgu": np.ascontiguousarray(np.stack(
            [w_gate.T.astype(bf).reshape(KT_H, 128, NB, BANK),
             w_up.T.astype(bf).reshape(KT_H, 128, NB, BANK)],
            axis=3).transpose(2, 1, 0, 3, 4)),
        # wd: [hc, k, 128, 512] from w_down.T [I, H]
        "wd": np.ascontiguousarray(
            w_down.T.astype(bf).reshape(NB, 128, H // 512, 512)
            .transpose(2, 0, 1, 3)),
    }
    x2 = x.reshape(NTOK_TOT, H)
    maps = []
    for c in range(NCORES):
        xT = x2[c * NTOK:(c + 1) * NTOK].T.copy()   # [H, NTOK]
        xrh, xrl = _split_r(xT)
        m = dict(shared)
        m["xTh"] = tile_kxn(xT.astype(bf), KT_H)
        m["xr"] = hl(xrh, xrl, KT_H)
        maps.append(m)
    return maps


def kernel(x, w_pred1, w_pred2, w_gate, w_up, w_down, balanced_bias,
           trace=False):
    x = np.asarray(x, dtype=np.float32)
    assert not np.any(np.asarray(balanced_bias)), \
        "kernel assumes balanced_bias == 0 (as produced by setup_inputs)"
    if "nc" not in _CACHE:
        _CACHE["nc"] = _build()
    nc = _CACHE["nc"]
    maps = _prep_inputs(x, np.asarray(w_pred1, np.float32),
                        np.asarray(w_pred2, np.float32),
                        np.asarray(w_gate, np.float32),
                        np.asarray(w_up, np.float32),
                        np.asarray(w_down, np.float32))
    res = run_bass_kernel_spmd(nc, maps, list(range(NCORES)), trace=trace)
    out = np.concatenate([res.results[c]["out"] for c in range(NCORES)], axis=0)
    out = out.reshape(x.shape[0], x.shape[1], H)
    if trace:
        _CACHE["last_result"] = res
    return out



# revision 4
# speedup vs baseline: 1.4462x; 1.4462x over previous
"""BalancedTopkMLP Trainium2 kernel: token-parallel across 8 NeuronCores.

reference:
  pred = sigmoid((x @ w_pred1.T) @ w_pred2.T)          [N, I]
  mask = per-bank (128ch) top-16 of |pred|+bias, binary  (bias == 0 here)
  out  = (mask*pred * silu(x@w_gate.T) * (x@w_up.T)) @ w_down.T

Sharding: tokens (B*S = 8192) split 8 ways; each core runs the full MLP on
its 1024 tokens with full weights (no collectives).

Numerics/speed: main matmuls (gate/up/down) use a 3-term fp8(e4m3) hi/lo
split executed with MatmulPerfMode.DoubleRow (2 k-tiles per instruction at
0.5 cycles/row -> 4x bf16 MAC throughput; 3 terms = 0.75x bf16 cost, rel
err ~0.11% < bf16's ~0.2%). The predictor (whose per-bank top-16 selection
needs ~1e-5 relative precision on z) uses an f32r hi term plus a packed
fp8 DoubleRow correction (xh@wl + xl@wh in ONE DR stream), z-noise ~7e-6.
Operands are pre-scaled by powers of 2 so fp8 values sit in e4m3's normal
range (sigma ~0.25-1); scales are unwound in activation/stt combines.

Layout trick: hi/lo fp8 pairs are interleaved with OPPOSITE sub-index
conventions for x vs w (x: [lo, hi], w: [hi, lo]), so one tile serves both
the main stream (pairs of hi along k) and the correction stream (hi x lo
cross terms via DR's positional k-tile pairing) with no duplication.
"""
import sys
import os
import numpy as np
import ml_dtypes

for _p in ("/opt/trn_rl_repo", os.path.expanduser("~/.axon_site/_ro/trn_rl_repo")):
    if os.path.isdir(_p) and _p not in sys.path:
        sys.path.insert(0, _p)

import concourse.bass as bass  # noqa: E402
import concourse.mybir as mybir  # noqa: E402
from concourse import bacc  # noqa: E402
from concourse.bass_utils import run_bass_kernel_spmd  # noqa: E402
from concourse.tile import TileContext  # noqa: E402
from concourse.masks import make_identity  # noqa: E402

BF16 = mybir.dt.bfloat16
F32R = mybir.dt.float32r
FP32 = mybir.dt.float32
FP8 = mybir.dt.float8e4
AF = mybir.ActivationFunctionType
OP = mybir.AluOpType
DR = mybir.MatmulPerfMode.DoubleRow
E4M3 = ml_dtypes.float8_e4m3

H = 4096
I = 11008
PD = 1024
BANK = 128
NB = I // BANK          # 86
NCORES = 8
NTOK = 8192 // NCORES   # 1024 per core
NTT = NTOK // 128       # 8 token tiles
CB = 4                  # banks per chunk
NCHUNK = (NB + CB - 1) // CB  # 22 (21x4 + 1x2)
KT_H = H // 128         # 32
KT_P = PD // 128        # 8
NEG = -1.0e30

_CACHE = {}


def _build():
    nc = bacc.Bacc("TRN2", target_bir_lowering=False, debug=False,
                   num_devices=NCORES)

    def din(name, shape, dt):
        return nc.declare_dram_parameter(name, list(shape), dt, isOutput=False)

    xh_d = din("xh", [128, KT_H, NTOK], F32R)        # f32r hi of xT
    x8_d = din("x8", [128, KT_H, 2, NTOK], FP8)      # (xh8, xl8*2^12)
    xab_d = din("xab", [128, KT_H, 2, NTOK], FP8)    # (xb, xa)
    w1h_d = din("w1h", [128, KT_H, PD], F32R)
    w1c_d = din("w1c", [128, KT_H, 2, PD], FP8)      # (w1l8*2^18, w1h8*2^6)
    w2h_d = din("w2h", [128, KT_P, I], F32R)
    w2c_d = din("w2c", [128, KT_P, 2, I], FP8)       # (w2l8*2^16, w2h8*2^4)
    wgu_d = din("wgu", [NB, 2, 128, KT_H, 2, BANK], FP8)  # (wa, wb) g/u
    wd_d = din("wd", [H // 512, NB, 128, 2, 512], FP8)    # (wda, wdb)
    out_d = nc.declare_dram_parameter("out", [NTOK, H], FP32, isOutput=True)

    from contextlib import ExitStack
    with TileContext(nc) as tc, ExitStack() as es:
        ep = es.enter_context
        constp = ep(tc.tile_pool(name="const", bufs=1))
        m8p = ep(tc.tile_pool(name="m8", bufs=8))
        dramp = ep(tc.tile_pool(name="dram", bufs=1, space="DRAM"))

        ident = constp.tile([128, 128], BF16)
        make_identity(nc, ident)
        hst = dramp.tile([128, NB, 2, NTOK], FP8, tag="hst")

        with tc.tile_pool(name="xab", bufs=1) as xabp, \
             tc.tile_pool(name="xpp", bufs=1) as xpp:
            xab = xabp.tile([128, KT_H, 2, NTOK], FP8, tag="xab")
            nc.gpsimd.dma_start(xab[:], xab_d[:])
            xph = xpp.tile([128, KT_P, NTOK], F32R, tag="xph")
            xp8 = xpp.tile([128, KT_P, 2, NTOK], FP8, tag="xp8")

            # ---------------- phase 1: xpT = w_pred1 @ x.T ----------------
            with tc.tile_pool(name="ps1", bufs=8, space="PSUM") as ps1, \
                 tc.tile_pool(name="xsp", bufs=2) as xsp, \
                 tc.tile_pool(name="w1p", bufs=4) as w1p, \
                 tc.tile_pool(name="xcp", bufs=8) as xcp, \
                 tc.tile_pool(name="xsb", bufs=4) as xsb:
                for grp in range(2):
                    mis = list(range(4 * grp, 4 * grp + 4))
                    # corr pass (fp8 DR): psum = 2^18*(w1l^T@xh + w1h^T@xl)
                    pcs = {}
                    for j in range(4):
                        for th in range(2):
                            pcs[j, th] = ps1.tile([128, 512], FP32, tag="p1",
                                                  name=f"pc{j}_{th}")
                    for q in range(KT_H // 4):
                        x8q = xsp.tile([128, 4, 2, NTOK], FP8, tag="xs",
                                       name="x8q")
                        nc.gpsimd.dma_start(x8q[:], x8_d[:, 4 * q:4 * q + 4])
                        for j, mi in enumerate(mis):
                            w1cq = w1p.tile([128, 4, 2, 128], FP8, tag="w1",
                                            name="w1cq")
                            nc.sync.dma_start(
                                w1cq[:],
                                w1c_d[:, 4 * q:4 * q + 4, :,
                                      mi * 128:(mi + 1) * 128])
                            for k in range(4):
                                for th in range(2):
                                    nc.tensor.matmul(
                                        pcs[j, th][:], w1cq[:, k],
                                        x8q[:, k, :, th * 512:th * 512 + 512],
                                        start=(q == 0 and k == 0),
                                        stop=(q == KT_H // 4 - 1 and k == 3),
                                        perf_mode=DR)
                    xpcs = {}
                    for j in range(4):
                        for th in range(2):
                            xpc = xcp.tile([128, 512], FP32, tag="xpc")
                            nc.scalar.activation(xpc[:], pcs[j, th][:],
                                                 AF.Copy, scale=2.0 ** -18)
                            xpcs[j, th] = xpc
                    # hi pass (f32r)
                    phs = {}
                    for j in range(4):
                        for th in range(2):
                            phs[j, th] = ps1.tile([128, 512], FP32, tag="p1",
                                                  name=f"ph{j}_{th}")
                    for q in range(KT_H // 4):
                        xhq = xsp.tile([128, 4, NTOK], F32R, tag="xs",
                                       name="xhq")
                        nc.gpsimd.dma_start(xhq[:], xh_d[:, 4 * q:4 * q + 4])
                        for j, mi in enumerate(mis):
                            w1hq = w1p.tile([128, 4, 128], F32R, tag="w1",
                                            name="w1hq")
                            nc.sync.dma_start(
                                w1hq[:],
                                w1h_d[:, 4 * q:4 * q + 4,
                                      mi * 128:(mi + 1) * 128])
                            for k in range(4):
                                for th in range(2):
                                    nc.tensor.matmul(
                                        phs[j, th][:], w1hq[:, k],
                                        xhq[:, k, th * 512:th * 512 + 512],
                                        start=(q == 0 and k == 0),
                                        stop=(q == KT_H // 4 - 1 and k == 3))
                    for j, mi in enumerate(mis):
                        for th in range(2):
                            ts_ = slice(th * 512, th * 512 + 512)
                            xps = xsb.tile([128, 512], FP32, tag="xps")
                            nc.vector.scalar_tensor_tensor(
                                xps[:], phs[j, th][:], 1.0, xpcs[j, th][:],
                                OP.mult, OP.add)
                            nc.scalar.activation(xph[:, mi, ts_], xps[:],
                                                 AF.Copy)
                            xpl = xsb.tile([128, 512], FP32, tag="xps",
                                           name="xpl")
                            nc.vector.tensor_tensor(
                                xpl[:], xps[:],
                                xph[:, mi, ts_].bitcast(FP32), OP.subtract)
                            nc.scalar.activation(
                                xp8[:, mi, 0, ts_],
                                xph[:, mi, ts_].bitcast(FP32), AF.Copy)
                            nc.scalar.activation(xp8[:, mi, 1, ts_], xpl[:],
                                                 AF.Copy, scale=4096.0)

            # ---------------- phase 2: chunk loop over I ----------------
            def _phase2(zps, gups, trps, w2hp, w2cp, zp, zapp, prp, zgp,
                        gfp, wgp, mtp, sgup, htp, hcbp):
                for ci in range(NCHUNK):
                    b0 = ci * CB
                    nb = min(CB, NB - b0)
                    cw = nb * BANK
                    c0 = b0 * BANK
                    nhf = (cw + 255) // 256
                    w2hts, w2cts = [], []
                    for hf in range(nhf):
                        hw_ = min(256, cw - hf * 256)
                        o0 = c0 + hf * 256
                        w2ht = w2hp.tile([128, KT_P, 256], F32R, tag="w2h")
                        nc.sync.dma_start(w2ht[:, :, :hw_],
                                          w2h_d[:, :, o0:o0 + hw_])
                        w2ct = w2cp.tile([128, KT_P, 2, 256], FP8, tag="w2c")
                        nc.sync.dma_start(w2ct[:, :, :, :hw_],
                                          w2c_d[:, :, :, o0:o0 + hw_])
                        w2hts.append((w2ht, hw_))
                        w2cts.append((w2ct, hw_))
                    pred = prp.tile([128, NTT, CB * BANK], BF16, tag="pred")
                    for tt in range(NTT):
                        tsl = slice(tt * 128, (tt + 1) * 128)
                        pzc = zps.tile([128, CB * BANK], FP32, tag="z",
                                       name="pzc")
                        for hf, (w2ct, hw_) in enumerate(w2cts):
                            osl = slice(hf * 256, hf * 256 + hw_)
                            for mi in range(KT_P):
                                nc.tensor.matmul(
                                    pzc[:, osl], xp8[:, mi, :, tsl],
                                    w2ct[:, mi, :, :hw_],
                                    start=(mi == 0), stop=(mi == KT_P - 1),
                                    perf_mode=DR)
                        z1 = zgp.tile([128, CB * BANK], BF16, tag="z1")
                        nc.scalar.activation(z1[:, :cw], pzc[:, :cw], AF.Copy,
                                             scale=2.0 ** -16)
                        pzh = zps.tile([128, CB * BANK], FP32, tag="z",
                                       name="pzh")
                        for hf, (w2ht, hw_) in enumerate(w2hts):
                            osl = slice(hf * 256, hf * 256 + hw_)
                            for mi in range(KT_P):
                                nc.tensor.matmul(
                                    pzh[:, osl], xph[:, mi, tsl],
                                    w2ht[:, mi, :hw_],
                                    start=(mi == 0), stop=(mi == KT_P - 1))
                        zt = zp.tile([128, CB * BANK], FP32, tag="zt")
                        nc.vector.scalar_tensor_tensor(
                            zt[:, :cw], pzh[:, :cw], 1.0, z1[:, :cw],
                            OP.mult, OP.add)
                        zap = zapp.tile([128, CB * BANK], FP32, tag="zap")
                        for b in range(nb):
                            bsl = slice(b * BANK, (b + 1) * BANK)
                            m8 = m8p.tile([128, 8], FP32, tag="m8")
                            nc.vector.max(m8[:], zt[:, bsl])
                            nc.vector.match_replace(
                                zap[:, bsl], in_to_replace=m8[:],
                                in_values=zt[:, bsl], imm_value=NEG)
                            m8b = m8p.tile([128, 8], FP32, tag="m8")
                            nc.vector.max(m8b[:], zap[:, bsl])
                            nc.vector.match_replace(
                                zap[:, bsl], in_to_replace=m8b[:],
                                in_values=zap[:, bsl], imm_value=NEG)
                        nc.scalar.activation(pred[:, tt, :cw], zt[:, :cw],
                                             AF.Sigmoid)
                        nc.vector.tensor_tensor(zap[:, :cw], zt[:, :cw],
                                                zap[:, :cw], OP.not_equal)
                        nc.vector.tensor_tensor(pred[:, tt, :cw],
                                                zap[:, :cw], pred[:, tt, :cw],
                                                OP.mult)

                    # gate/up (PE-heavy, independent of pred) then per-bank
                    # transposes + h assembly
                    hcbs = []
                    for b in range(nb):
                        it = b0 + b
                        wgt = wgp.tile([128, KT_H, 2, BANK], FP8, tag="wg",
                                       name="wgt")
                        nc.gpsimd.dma_start(wgt[:], wgu_d[it, 0])
                        wut = wgp.tile([128, KT_H, 2, BANK], FP8, tag="wg",
                                       name="wut")
                        nc.gpsimd.dma_start(wut[:], wgu_d[it, 1])
                        sgb = sgup.tile([128, NTOK], BF16, tag="sg")
                        uub = sgup.tile([128, NTOK], BF16, tag="uu")
                        for wt, ob, act in ((wgt, sgb, AF.Silu),
                                            (wut, uub, AF.Copy)):
                            for th in range(2):
                                ts_ = slice(th * 512, th * 512 + 512)
                                pc_ = gups.tile([128, 512], FP32, tag="gu",
                                                name="pc")
                                for j in range(KT_H):
                                    nc.tensor.matmul(
                                        pc_[:], wt[:, j], xab[:, j, :, ts_],
                                        start=(j == 0), stop=(j == KT_H - 1),
                                        perf_mode=DR)
                                g1 = zgp.tile([128, 512], BF16, tag="g1")
                                nc.scalar.activation(g1[:], pc_[:], AF.Copy,
                                                     scale=2.0 ** -10)
                                pm_ = gups.tile([128, 512], FP32, tag="gu",
                                                name="pm")
                                for jp in range(KT_H // 2):
                                    nc.tensor.matmul(
                                        pm_[:], wt[:, 2 * jp:2 * jp + 2, 0, :],
                                        xab[:, 2 * jp:2 * jp + 2, 1, ts_],
                                        start=(jp == 0),
                                        stop=(jp == KT_H // 2 - 1),
                                        perf_mode=DR)
                                gf = gfp.tile([128, 512], FP32, tag="gf")
                                nc.vector.scalar_tensor_tensor(
                                    gf[:], pm_[:], 2.0 ** -6, g1[:],
                                    OP.mult, OP.add)
                                nc.scalar.activation(ob[:, ts_], gf[:], act)
                        # transposes for this bank (pred ready by now)
                        mpT = mtp.tile([128, NTOK], BF16, tag="mpT")
                        for tt in range(NTT):
                            tp = trps.tile([128, 128], BF16, tag="tr")
                            nc.tensor.transpose(
                                tp[:], pred[:, tt, b * BANK:(b + 1) * BANK],
                                ident[:])
                            nc.scalar.activation(
                                mpT[:, tt * 128:(tt + 1) * 128], tp[:],
                                AF.Copy)
                        htb = htp.tile([128, NTOK], BF16, tag="ht")
                        nc.vector.tensor_tensor(htb[:], mpT[:], sgb[:],
                                                OP.mult)
                        nc.vector.tensor_tensor(htb[:], htb[:], uub[:],
                                                OP.mult)
                        hcb = hcbp.tile([128, 2, NTOK], FP8, tag="hcb")
                        nc.scalar.activation(hcb[:, 1, :], htb[:], AF.Copy)
                        hdb = htp.tile([128, NTOK], BF16, tag="hd")
                        nc.vector.scalar_tensor_tensor(
                            hdb[:], htb[:], 1.0, hcb[:, 1, :],
                            OP.mult, OP.subtract)
                        nc.scalar.activation(hcb[:, 0, :], hdb[:], AF.Copy,
                                             scale=16.0)
                        nc.sync.dma_start(hst[:, it], hcb[:])
                        hcbs.append(hcb)

            with ExitStack() as es2:
                ep2 = es2.enter_context
                _phase2(
                    ep2(tc.tile_pool(name="zps", bufs=3, space="PSUM")),
                    ep2(tc.tile_pool(name="gups", bufs=4, space="PSUM")),
                    ep2(tc.tile_pool(name="trps", bufs=1, space="PSUM")),
                    ep2(tc.tile_pool(name="w2hp", bufs=3)),
                    ep2(tc.tile_pool(name="w2cp", bufs=3)),
                    ep2(tc.tile_pool(name="zp", bufs=1)),
                    ep2(tc.tile_pool(name="zapp", bufs=1)),
                    ep2(tc.tile_pool(name="prp", bufs=1)),
                    ep2(tc.tile_pool(name="zgp", bufs=2)),
                    ep2(tc.tile_pool(name="gfp", bufs=2)),
                    ep2(tc.tile_pool(name="wgp", bufs=2)),
                    ep2(tc.tile_pool(name="mtp", bufs=2)),
                    ep2(tc.tile_pool(name="sgup", bufs=2)),
                    ep2(tc.tile_pool(name="htp", bufs=1)),
                    ep2(tc.tile_pool(name="hcbp", bufs=2)))

        # ---------------- phase 3: out = hT.T @ w_downT ----------------
        GRPS = [(g0, min(8, NB - g0)) for g0 in range(0, NB, 8)]

        def _phase3(dnp, wdp, hp3, outp, c1p):
            for hc in range(H // 512):
                for ttg in range(2):
                    tg = slice(ttg * 512, ttg * 512 + 512)
                    pcs = [dnp.tile([128, 512], FP32, tag="dn",
                                    name=f"dc{lt}") for lt in range(4)]
                    pms = [dnp.tile([128, 512], FP32, tag="dn",
                                    name=f"dm{lt}") for lt in range(4)]
                    for g, (g0, gk) in enumerate(GRPS):
                        wdt = wdp.tile([128, 8, 2, 512], FP8, tag="wd")
                        nc.gpsimd.dma_start(
                            wdt[:, :gk], wd_d[hc, g0:g0 + gk].rearrange(
                                "k p i n -> p k i n"))
                        ht3 = hp3.tile([128, 8, 2, 512], FP8, tag="h3")
                        nc.sync.dma_start(ht3[:, :gk], hst[:, g0:g0 + gk, :, tg])
                        last = (g == len(GRPS) - 1)
                        for lt in range(4):
                            lsl = slice(lt * 128, (lt + 1) * 128)
                            for j in range(gk):
                                nc.tensor.matmul(
                                    pcs[lt][:], ht3[:, j, :, lsl], wdt[:, j],
                                    start=(g == 0 and j == 0),
                                    stop=(last and j == gk - 1), perf_mode=DR)
                            for jp in range(gk // 2):
                                nc.tensor.matmul(
                                    pms[lt][:],
                                    ht3[:, 2 * jp:2 * jp + 2, 1, lsl],
                                    wdt[:, 2 * jp:2 * jp + 2, 0, :],
                                    start=(g == 0 and jp == 0),
                                    stop=(last and jp == gk // 2 - 1),
                                    perf_mode=DR)
                    for lt in range(4):
                        c1 = c1p.tile([128, 512], BF16, tag="c1")
                        nc.scalar.activation(c1[:], pcs[lt][:], AF.Copy,
                                             scale=2.0 ** -10)
                        osb = outp.tile([128, 512], FP32, tag="os")
                        nc.vector.scalar_tensor_tensor(
                            osb[:], pms[lt][:], 2.0 ** -6, c1[:],
                            OP.mult, OP.add)
                        t0 = (ttg * 4 + lt) * 128
                        nc.sync.dma_start(
                            out_d[t0:t0 + 128, hc * 512:(hc + 1) * 512],
                            osb[:])

        with ExitStack() as es3:
            ep3 = es3.enter_context
            _phase3(ep3(tc.tile_pool(name="dnp", bufs=8, space="PSUM")),
                    ep3(tc.tile_pool(name="wdp", bufs=2)),
                    ep3(tc.tile_pool(name="hp3", bufs=2)),
                    ep3(tc.tile_pool(name="outp", bufs=4)),
                    ep3(tc.tile_pool(name="c1p", bufs=4)))

    nc.compile()
    return nc


def _rne12(a):
    """float32r rounding: round-to-nearest-even keeping 11 explicit mantissa
    bits (drops 12 low bits), as measured on TRN2 via identity matmul."""
    v = np.ascontiguousarray(a, np.float32).view(np.uint32)
    add = np.uint32((1 << 11) - 1)
    lsb = (v >> np.uint32(12)) & np.uint32(1)
    return ((v + add + lsb) & np.uint32(0xFFFFF000)).view(np.float32)


def _til(a, kt):
    """[K, N] -> [128, kt, N] partition-first tiling."""
    K, N = a.shape
    return np.ascontiguousarray(a.reshape(kt, 128, N).transpose(1, 0, 2))


def _til2(a, b, kt):
    """two [K, N] -> [128, kt, 2, N] interleaved."""
    K, N = a.shape
    s = np.stack([a.reshape(kt, 128, N), b.reshape(kt, 128, N)], axis=2)
    return np.ascontiguousarray(s.transpose(1, 0, 2, 3))


def _prep_inputs(x, w_pred1, w_pred2, w_gate, w_up, w_down):
    def q(a):
        return a.astype(E4M3)

    def qf(a):
        return a.astype(E4M3).astype(np.float32)

    w1 = w_pred1.T.copy()                     # [H, PD]
    w1h = _rne12(w1)
    w1l = w1 - w1h
    w2 = w_pred2.T.copy()                     # [PD, I]
    w2h = _rne12(w2)
    w2l = w2 - w2h

    def wgu_pack(w):                          # w: [I, H] fp32
        ws = (w.T * 64.0).astype(np.float32)  # [H, I]
        wa = qf(ws)
        wbq = q((ws - wa) * 16.0)
        waq = q(ws)
        a = waq.reshape(KT_H, 128, NB, BANK)
        b = wbq.reshape(KT_H, 128, NB, BANK)
        s = np.stack([a, b], axis=3)          # [kt,128,NB,2,BANK]
        return s.transpose(2, 1, 0, 3, 4)     # [NB,128,kt,2,BANK]

    wD = (w_down.T * 64.0).astype(np.float32)  # [I, H]
    wda = qf(wD)
    wdaq = q(wD)
    wdbq = q((wD - wda) * 16.0)
    a = wdaq.reshape(NB, 128, H // 512, 512)
    b = wdbq.reshape(NB, 128, H // 512, 512)
    wd = np.ascontiguousarray(
        np.stack([a, b], axis=3).transpose(2, 0, 1, 3, 4))  # [hc,NB,128,2,512]

    shared = {
        "w1h": _til(w1h, KT_H),
        "w1c": _til2(q(w1l * 2.0 ** 18), q(w1h * 2.0 ** 6), KT_H),
        "w2h": _til(w2h, KT_P),
        "w2c": _til2(q(w2l * 2.0 ** 16), q(w2h * 2.0 ** 4), KT_P),
        "wgu": np.ascontiguousarray(
            np.stack([wgu_pack(w_gate), wgu_pack(w_up)], axis=1)),
        "wd": wd,
    }
    x2 = x.reshape(NCORES * NTOK, H)
    maps = []
    for c in range(NCORES):
        xT = x2[c * NTOK:(c + 1) * NTOK].T.copy()   # [H, NTOK]
        xh = _rne12(xT)
        xl = xT - xh
        xa = qf(xT)
        m = dict(shared)
        m["xh"] = _til(xh, KT_H)
        m["x8"] = _til2(q(xh), q(xl * 4096.0), KT_H)
        m["xab"] = _til2(q((xT - xa) * 16.0), q(xT), KT_H)
        maps.append(m)
    return maps


def kernel(x, w_pred1, w_pred2, w_gate, w_up, w_down, balanced_bias,
           trace=False):
    x = np.asarray(x, dtype=np.float32)
    assert not np.any(np.asarray(balanced_bias)), \
        "kernel assumes balanced_bias == 0 (as produced by setup_inputs)"
    if "nc" not in _CACHE:
        _CACHE["nc"] = _build()
    nc = _CACHE["nc"]
    maps = _prep_inputs(x, np.asarray(w_pred1, np.float32),
                        np.asarray(w_pred2, np.float32),
                        np.asarray(w_gate, np.float32),
                        np.asarray(w_up, np.float32),
                        np.asarray(w_down, np.float32))
    res = run_bass_kernel_spmd(nc, maps, list(range(NCORES)), trace=trace)
    out = np.concatenate([res.results[c]["out"] for c in range(NCORES)], axis=0)
    out = out.reshape(x.shape[0], x.shape[1], H)
    if trace:
        _CACHE["last_result"] = res
    return out


# revision 5
# speedup vs baseline: 1.4633x; 1.0118x over previous
"""BalancedTopkMLP Trainium2 kernel: token-parallel across 8 NeuronCores.

reference:
  pred = sigmoid((x @ w_pred1.T) @ w_pred2.T)          [N, I]
  mask = per-bank (128ch) top-16 of |pred|+bias, binary  (bias == 0 here)
  out  = (mask*pred * silu(x@w_gate.T) * (x@w_up.T)) @ w_down.T

Sharding: tokens (B*S = 8192) split 8 ways; each core runs the full MLP on
its 1024 tokens with full weights (no collectives).

Numerics/speed: main matmuls (gate/up/down) use a 3-term fp8(e4m3) hi/lo
split executed with MatmulPerfMode.DoubleRow (2 k-tiles per instruction at
0.5 cycles/row -> 4x bf16 MAC throughput as modeled; 3 terms = 0.75x bf16
cost, rel err ~0.11%). The predictor (whose per-bank top-16 selection
needs ~1e-5 relative precision on z) uses an f32r hi term plus a packed
fp8 DoubleRow correction (xh@wl + xl@wh in ONE DR stream), z-noise ~7e-6.

All terms of one output accumulate in a SINGLE psum bank: operands are
pre-scaled by powers of 2 (exact in fp8/f32r) so every term lands at the
same psum scale, unwound by the final activation's scale. Main-stream hi
weights are shipped twice (wa at 2^6 for the correction pairing, wa2 at
2^10 matching the correction products), costing +50% weight DMA but
removing all combine ops and halving psum pressure.

Layout trick: hi/lo fp8 pairs are interleaved with OPPOSITE sub-index
conventions for x vs w (x: [lo, hi], w: [hi, lo, hi*16]), so one tile
serves both the main stream (pairs of hi along k) and the correction
stream (hi x lo cross terms via DR's positional k-tile pairing).
"""
import sys
import os
import numpy as np
import ml_dtypes

for _p in ("/opt/trn_rl_repo", os.path.expanduser("~/.axon_site/_ro/trn_rl_repo")):
    if os.path.isdir(_p) and _p not in sys.path:
        sys.path.insert(0, _p)

import concourse.bass as bass  # noqa: E402
import concourse.mybir as mybir  # noqa: E402
from concourse import bacc  # noqa: E402
from concourse.bass_utils import run_bass_kernel_spmd  # noqa: E402
from concourse.tile import TileContext  # noqa: E402
from concourse.masks import make_identity  # noqa: E402

BF16 = mybir.dt.bfloat16
F32R = mybir.dt.float32r
FP32 = mybir.dt.float32
FP8 = mybir.dt.float8e4
AF = mybir.ActivationFunctionType
OP = mybir.AluOpType
DR = mybir.MatmulPerfMode.DoubleRow
E4M3 = ml_dtypes.float8_e4m3

H = 4096
I = 11008
PD = 1024
BANK = 128
NB = I // BANK          # 86
NCORES = 8
NTOK = 8192 // NCORES   # 1024 per core
NTT = NTOK // 128       # 8 token tiles
CB = 4                  # banks per chunk
NCHUNK = (NB + CB - 1) // CB  # 22 (21x4 + 1x2)
KT_H = H // 128         # 32
KT_P = PD // 128        # 8
QG = 8                  # phase-1 k-tiles per streamed group
NEG = -1.0e30

_CACHE = {}


def _build():
    nc = bacc.Bacc("TRN2", target_bir_lowering=False, debug=False,
                   num_devices=NCORES)

    def din(name, shape, dt):
        return nc.declare_dram_parameter(name, list(shape), dt, isOutput=False)

    xh_d = din("xh", [128, KT_H, NTOK], F32R)        # f32r hi of xT
    x8_d = din("x8", [128, KT_H, 2, NTOK], FP8)      # (xh8, xl8*2^12)
    xab_d = din("xab", [128, KT_H, 2, NTOK], FP8)    # (xb, xa)
    w1h_d = din("w1h", [128, KT_H, PD], F32R)        # w1h * 2^18
    w1c_d = din("w1c", [128, KT_H, 2, PD], FP8)      # (w1l8*2^18, w1h8*2^6)
    w2h_d = din("w2h", [128, KT_P, I], F32R)         # w2h * 2^16
    w2c_d = din("w2c", [128, KT_P, 2, I], FP8)       # (w2l8*2^16, w2h8*2^4)
    wgu_d = din("wgu", [NB, 2, 128, KT_H, 3, BANK], FP8)  # (wa, wb, wa2)
    wd_d = din("wd", [H // 512, NB, 128, 3, 512], FP8)    # (wda, wdb, wda2)
    out_d = nc.declare_dram_parameter("out", [NTOK, H], FP32, isOutput=True)

    from contextlib import ExitStack
    with TileContext(nc) as tc, ExitStack() as es:
        ep = es.enter_context
        constp = ep(tc.tile_pool(name="const", bufs=1))
        m8p = ep(tc.tile_pool(name="m8", bufs=8))
        dramp = ep(tc.tile_pool(name="dram", bufs=1, space="DRAM"))

        ident = constp.tile([128, 128], BF16)
        make_identity(nc, ident)
        hst = dramp.tile([128, NB, 2, NTOK], FP8, tag="hst")

        def _phase1(ps1, x8p, xhp, w1cp, w1hp, xsb, xph, xp8):
            for th in range(2):
                ts_ = slice(th * 512, th * 512 + 512)
                pss = [ps1.tile([128, 512], FP32, tag="p1", name=f"ps{mi}")
                       for mi in range(KT_P)]
                for qq in range(KT_H // QG):
                    k0 = qq * QG
                    x8q = x8p.tile([128, QG, 2, 512], FP8, tag="x8q")
                    nc.gpsimd.dma_start(x8q[:], x8_d[:, k0:k0 + QG, :, ts_])
                    xhq = xhp.tile([128, QG, 512], F32R, tag="xhq")
                    nc.gpsimd.dma_start(xhq[:], xh_d[:, k0:k0 + QG, ts_])
                    for mi in range(KT_P):
                        msl = slice(mi * 128, (mi + 1) * 128)
                        w1cq = w1cp.tile([128, QG, 2, 128], FP8, tag="w1c")
                        nc.gpsimd.dma_start(w1cq[:],
                                            w1c_d[:, k0:k0 + QG, :, msl])
                        w1hq = w1hp.tile([128, QG, 128], F32R, tag="w1h")
                        nc.sync.dma_start(w1hq[:], w1h_d[:, k0:k0 + QG, msl])
                        for k in range(QG):
                            nc.tensor.matmul(
                                pss[mi][:], w1cq[:, k], x8q[:, k],
                                start=(qq == 0 and k == 0), stop=False,
                                perf_mode=DR)
                            nc.tensor.matmul(
                                pss[mi][:], w1hq[:, k], xhq[:, k],
                                start=False,
                                stop=(qq == KT_H // QG - 1 and k == QG - 1))
                for mi in range(KT_P):
                    xps = xsb.tile([128, 512], FP32, tag="xps")
                    nc.scalar.activation(xps[:], pss[mi][:], AF.Copy,
                                         scale=2.0 ** -18)
                    nc.scalar.activation(xph[:, mi, ts_], xps[:], AF.Copy)
                    xpl = xsb.tile([128, 512], FP32, tag="xps", name="xpl")
                    nc.vector.tensor_tensor(xpl[:], xps[:],
                                            xph[:, mi, ts_].bitcast(FP32),
                                            OP.subtract)
                    nc.scalar.activation(xp8[:, mi, 0, ts_],
                                         xph[:, mi, ts_].bitcast(FP32),
                                         AF.Copy)
                    nc.scalar.activation(xp8[:, mi, 1, ts_], xpl[:],
                                         AF.Copy, scale=4096.0)

        def _phase2(zps, gups, trps, w2hp, w2cp, zp, zapp, prp, wgp, mtp,
                    sgup, htp, hcbp, xab, xph, xp8):
            for ci in range(NCHUNK):
                b0 = ci * CB
                nb = min(CB, NB - b0)
                cw = nb * BANK
                c0 = b0 * BANK
                nhf = (cw + 255) // 256
                w2hts, w2cts = [], []
                for hf in range(nhf):
                    hw_ = min(256, cw - hf * 256)
                    o0 = c0 + hf * 256
                    w2ht = w2hp.tile([128, KT_P, 256], F32R, tag="w2h")
                    nc.sync.dma_start(w2ht[:, :, :hw_],
                                      w2h_d[:, :, o0:o0 + hw_])
                    w2ct = w2cp.tile([128, KT_P, 2, 256], FP8, tag="w2c")
                    nc.sync.dma_start(w2ct[:, :, :, :hw_],
                                      w2c_d[:, :, :, o0:o0 + hw_])
                    w2hts.append((w2ht, hw_))
                    w2cts.append((w2ct, hw_))
                pred = prp.tile([128, NTT, CB * BANK], BF16, tag="pred")
                for tt in range(NTT):
                    tsl = slice(tt * 128, (tt + 1) * 128)
                    pz = zps.tile([128, CB * BANK], FP32, tag="z")
                    for hf in range(nhf):
                        w2ct, hw_ = w2cts[hf]
                        w2ht, _ = w2hts[hf]
                        osl = slice(hf * 256, hf * 256 + hw_)
                        for mi in range(KT_P):
                            nc.tensor.matmul(
                                pz[:, osl], xp8[:, mi, :, tsl],
                                w2ct[:, mi, :, :hw_],
                                start=(mi == 0), stop=False, perf_mode=DR)
                        for mi in range(KT_P):
                            nc.tensor.matmul(
                                pz[:, osl], xph[:, mi, tsl],
                                w2ht[:, mi, :hw_],
                                start=False, stop=(mi == KT_P - 1))
                    zt = zp.tile([128, CB * BANK], FP32, tag="zt")
                    nc.scalar.activation(zt[:, :cw], pz[:, :cw], AF.Copy,
                                         scale=2.0 ** -16)
                    zap = zapp.tile([128, CB * BANK], FP32, tag="zap")
                    for b in range(nb):
                        bsl = slice(b * BANK, (b + 1) * BANK)
                        m8 = m8p.tile([128, 8], FP32, tag="m8")
                        nc.vector.max(m8[:], zt[:, bsl])
                        nc.vector.match_replace(
                            zap[:, bsl], in_to_replace=m8[:],
                            in_values=zt[:, bsl], imm_value=NEG)
                        m8b = m8p.tile([128, 8], FP32, tag="m8")
                        nc.vector.max(m8b[:], zap[:, bsl])
                        nc.vector.match_replace(
                            zap[:, bsl], in_to_replace=m8b[:],
                            in_values=zap[:, bsl], imm_value=NEG)
                    nc.scalar.activation(pred[:, tt, :cw], zt[:, :cw],
                                         AF.Sigmoid)
                    nc.vector.tensor_tensor(zap[:, :cw], zt[:, :cw],
                                            zap[:, :cw], OP.not_equal)
                    nc.vector.tensor_tensor(pred[:, tt, :cw], zap[:, :cw],
                                            pred[:, tt, :cw], OP.mult)

                # gate/up (PE-heavy, independent of pred), then per-bank
                # transposes + h assembly
                for b in range(nb):
                    it = b0 + b
                    wgt = wgp.tile([128, KT_H, 3, BANK], FP8, tag="wg",
                                   name="wgt")
                    nc.gpsimd.dma_start(wgt[:], wgu_d[it, 0])
                    wut = wgp.tile([128, KT_H, 3, BANK], FP8, tag="wg",
                                   name="wut")
                    nc.gpsimd.dma_start(wut[:], wgu_d[it, 1])
                    sgb = sgup.tile([128, NTOK], BF16, tag="sg")
                    uub = sgup.tile([128, NTOK], BF16, tag="uu")
                    for wt, ob, act in ((wgt, sgb, AF.Silu),
                                        (wut, uub, AF.Copy)):
                        for th in range(2):
                            ts_ = slice(th * 512, th * 512 + 512)
                            pc_ = gups.tile([128, 512], FP32, tag="gu")
                            for j in range(KT_H):
                                nc.tensor.matmul(
                                    pc_[:], wt[:, j, 0:2, :],
                                    xab[:, j, :, ts_],
                                    start=(j == 0), stop=False, perf_mode=DR)
                            for jp in range(KT_H // 2):
                                nc.tensor.matmul(
                                    pc_[:], wt[:, 2 * jp:2 * jp + 2, 2, :],
                                    xab[:, 2 * jp:2 * jp + 2, 1, ts_],
                                    start=False,
                                    stop=(jp == KT_H // 2 - 1), perf_mode=DR)
                            nc.scalar.activation(ob[:, ts_], pc_[:], act,
                                                 scale=2.0 ** -10)
                    # transposes for this bank (pred ready by now)
                    mpT = mtp.tile([128, NTOK], BF16, tag="mpT")
                    for tt in range(NTT):
                        tp = trps.tile([128, 128], BF16, tag="tr")
                        nc.tensor.transpose(
                            tp[:], pred[:, tt, b * BANK:(b + 1) * BANK],
                            ident[:])
                        nc.scalar.activation(
                            mpT[:, tt * 128:(tt + 1) * 128], tp[:], AF.Copy)
                    htb = htp.tile([128, NTOK], BF16, tag="ht")
                    nc.vector.tensor_tensor(htb[:], mpT[:], sgb[:], OP.mult)
                    nc.vector.tensor_tensor(htb[:], htb[:], uub[:], OP.mult)
                    hcb = hcbp.tile([128, 2, NTOK], FP8, tag="hcb")
                    nc.scalar.activation(hcb[:, 1, :], htb[:], AF.Copy)
                    hdb = htp.tile([128, NTOK], BF16, tag="hd")
                    nc.vector.scalar_tensor_tensor(
                        hdb[:], htb[:], 1.0, hcb[:, 1, :],
                        OP.mult, OP.subtract)
                    nc.scalar.activation(hcb[:, 0, :], hdb[:], AF.Copy,
                                         scale=16.0)
                    nc.sync.dma_start(hst[:, it], hcb[:])

        GRPS = [(g0, min(8, NB - g0)) for g0 in range(0, NB, 8)]

        def _phase3(dnp, wdp, hp3, outp):
            for hc in range(H // 512):
                pss = [dnp.tile([128, 512], FP32, tag="dn", name=f"d{tt}")
                       for tt in range(NTT)]
                for g, (g0, gk) in enumerate(GRPS):
                    wdt = wdp.tile([128, 8, 3, 512], FP8, tag="wd")
                    nc.gpsimd.dma_start(
                        wdt[:, :gk], wd_d[hc, g0:g0 + gk].rearrange(
                            "k p i n -> p k i n"))
                    ht3 = hp3.tile([128, 8, 2, NTOK], FP8, tag="h3")
                    nc.sync.dma_start(ht3[:, :gk], hst[:, g0:g0 + gk])
                    last = (g == len(GRPS) - 1)
                    for tt in range(NTT):
                        lsl = slice(tt * 128, (tt + 1) * 128)
                        for j in range(gk):
                            nc.tensor.matmul(
                                pss[tt][:], ht3[:, j, :, lsl],
                                wdt[:, j, 0:2, :],
                                start=(g == 0 and j == 0), stop=False,
                                perf_mode=DR)
                        for jp in range(gk // 2):
                            nc.tensor.matmul(
                                pss[tt][:],
                                ht3[:, 2 * jp:2 * jp + 2, 1, lsl],
                                wdt[:, 2 * jp:2 * jp + 2, 2, :],
                                start=False,
                                stop=(last and jp == gk // 2 - 1),
                                perf_mode=DR)
                for tt in range(NTT):
                    osb = outp.tile([128, 512], FP32, tag="os")
                    nc.scalar.activation(osb[:], pss[tt][:], AF.Copy,
                                         scale=2.0 ** -10)
                    nc.sync.dma_start(
                        out_d[tt * 128:(tt + 1) * 128,
                              hc * 512:(hc + 1) * 512], osb[:])

        with tc.tile_pool(name="xab", bufs=1) as xabp, \
             tc.tile_pool(name="xpp", bufs=1) as xpp:
            xab = xabp.tile([128, KT_H, 2, NTOK], FP8, tag="xab")
            nc.gpsimd.dma_start(xab[:], xab_d[:])
            xph = xpp.tile([128, KT_P, NTOK], F32R, tag="xph")
            xp8 = xpp.tile([128, KT_P, 2, NTOK], FP8, tag="xp8")

            with ExitStack() as es1:
                ep1 = es1.enter_context
                _phase1(ep1(tc.tile_pool(name="ps1", bufs=8, space="PSUM")),
                        ep1(tc.tile_pool(name="x8p", bufs=2)),
                        ep1(tc.tile_pool(name="xhp", bufs=2)),
                        ep1(tc.tile_pool(name="w1cp", bufs=4)),
                        ep1(tc.tile_pool(name="w1hp", bufs=4)),
                        ep1(tc.tile_pool(name="xsb", bufs=4)),
                        xph, xp8)

            with ExitStack() as es2:
                ep2 = es2.enter_context
                _phase2(ep2(tc.tile_pool(name="zps", bufs=3, space="PSUM")),
                        ep2(tc.tile_pool(name="gups", bufs=4, space="PSUM")),
                        ep2(tc.tile_pool(name="trps", bufs=1, space="PSUM")),
                        ep2(tc.tile_pool(name="w2hp", bufs=3)),
                        ep2(tc.tile_pool(name="w2cp", bufs=3)),
                        ep2(tc.tile_pool(name="zp", bufs=1)),
                        ep2(tc.tile_pool(name="zapp", bufs=1)),
                        ep2(tc.tile_pool(name="prp", bufs=1)),
                        ep2(tc.tile_pool(name="wgp", bufs=2)),
                        ep2(tc.tile_pool(name="mtp", bufs=2)),
                        ep2(tc.tile_pool(name="sgup", bufs=2)),
                        ep2(tc.tile_pool(name="htp", bufs=1)),
                        ep2(tc.tile_pool(name="hcbp", bufs=2)),
                        xab, xph, xp8)

        with ExitStack() as es3:
            ep3 = es3.enter_context
            _phase3(ep3(tc.tile_pool(name="dnp", bufs=8, space="PSUM")),
                    ep3(tc.tile_pool(name="wdp", bufs=2)),
                    ep3(tc.tile_pool(name="hp3", bufs=2)),
                    ep3(tc.tile_pool(name="outp", bufs=4)))

    nc.compile()
    return nc


def _rne12(a):
    """float32r rounding: round-to-nearest-even keeping 11 explicit mantissa
    bits (drops 12 low bits), as measured on TRN2 via identity matmul."""
    v = np.ascontiguousarray(a, np.float32).view(np.uint32)
    add = np.uint32((1 << 11) - 1)
    lsb = (v >> np.uint32(12)) & np.uint32(1)
    return ((v + add + lsb) & np.uint32(0xFFFFF000)).view(np.float32)


def _til(a, kt):
    """[K, N] -> [128, kt, N] partition-first tiling."""
    K, N = a.shape
    return np.ascontiguousarray(a.reshape(kt, 128, N).transpose(1, 0, 2))


def _til2(a, b, kt):
    """two [K, N] -> [128, kt, 2, N] interleaved."""
    K, N = a.shape
    s = np.stack([a.reshape(kt, 128, N), b.reshape(kt, 128, N)], axis=2)
    return np.ascontiguousarray(s.transpose(1, 0, 2, 3))


def _prep_inputs(x, w_pred1, w_pred2, w_gate, w_up, w_down):
    def q(a):
        return a.astype(E4M3)

    def qf(a):
        return a.astype(E4M3).astype(np.float32)

    w1 = w_pred1.T.copy()                     # [H, PD]
    w1h = _rne12(w1)
    w1l = w1 - w1h
    w2 = w_pred2.T.copy()                     # [PD, I]
    w2h = _rne12(w2)
    w2l = w2 - w2h

    def wgu_pack(w):                          # w: [I, H] fp32
        ws = (w.T * 64.0).astype(np.float32)  # [H, I]
        wa = qf(ws)
        waq = q(ws)
        wbq = q((ws - wa) * 16.0)
        wa2q = q(ws * 16.0)
        a = waq.reshape(KT_H, 128, NB, BANK)
        b = wbq.reshape(KT_H, 128, NB, BANK)
        c = wa2q.reshape(KT_H, 128, NB, BANK)
        s = np.stack([a, b, c], axis=3)       # [kt,128,NB,3,BANK]
        return s.transpose(2, 1, 0, 3, 4)     # [NB,128,kt,3,BANK]

    wD = (w_down.T * 64.0).astype(np.float32)  # [I, H]
    wda = qf(wD)
    a = q(wD).reshape(NB, 128, H // 512, 512)
    b = q((wD - wda) * 16.0).reshape(NB, 128, H // 512, 512)
    c = q(wD * 16.0).reshape(NB, 128, H // 512, 512)
    wd = np.ascontiguousarray(
        np.stack([a, b, c], axis=3).transpose(2, 0, 1, 3, 4))  # [hc,NB,128,3,512]

    shared = {
        "w1h": _til(w1h * 2.0 ** 18, KT_H),
        "w1c": _til2(q(w1l * 2.0 ** 18), q(w1h * 2.0 ** 6), KT_H),
        "w2h": _til(w2h * 2.0 ** 16, KT_P),
        "w2c": _til2(q(w2l * 2.0 ** 16), q(w2h * 2.0 ** 4), KT_P),
        "wgu": np.ascontiguousarray(
            np.stack([wgu_pack(w_gate), wgu_pack(w_up)], axis=1)),
        "wd": wd,
    }
    x2 = x.reshape(NCORES * NTOK, H)
    maps = []
    for c_ in range(NCORES):
        xT = x2[c_ * NTOK:(c_ + 1) * NTOK].T.copy()   # [H, NTOK]
        xh = _rne12(xT)
        xl = xT - xh
        xa = qf(xT)
        m = dict(shared)
        m["xh"] = _til(xh, KT_H)
        m["x8"] = _til2(q(xh), q(xl * 4096.0), KT_H)
        m["xab"] = _til2(q((xT - xa) * 16.0), q(xT), KT_H)
        maps.append(m)
    return maps


def kernel(x, w_pred1, w_pred2, w_gate, w_up, w_down, balanced_bias,
           trace=False):
    x = np.asarray(x, dtype=np.float32)
    assert not np.any(np.asarray(balanced_bias)), \
        "kernel assumes balanced_bias == 0 (as produced by setup_inputs)"
    if "nc" not in _CACHE:
        _CACHE["nc"] = _build()
    nc = _CACHE["nc"]
    maps = _prep_inputs(x, np.asarray(w_pred1, np.float32),
                        np.asarray(w_pred2, np.float32),
                        np.asarray(w_gate, np.float32),
                        np.asarray(w_up, np.float32),
                        np.asarray(w_down, np.float32))
    res = run_bass_kernel_spmd(nc, maps, list(range(NCORES)), trace=trace)
    out = np.concatenate([res.results[c]["out"] for c in range(NCORES)], axis=0)
    out = out.reshape(x.shape[0], x.shape[1], H)
    if trace:
        _CACHE["last_result"] = res
    return out
